# revision 7
# baseline (speedup 1.0000x reference)
"""Causal single-head attention on 8 TRN2 NeuronCores.

Problem (hardcoded): x [4, 2048, 1024] f32; Wk, Wq, Wv [1024, 1024] f32.
  q = x @ Wk.T ; k = x @ Wq.T ; v = x @ Wv.T        (note ref's q/k weight swap)
  out = softmax(mask(q @ k.T) / sqrt(1024)) @ v

Sharding: 2 cores per batch. Core h of a batch computes 1024 queries as two
512-query tiles: tile A with a 1024-key context, tile B with a 2048-key
context.  h=0 owns query blocks [0:512) + [1536:2048), h=1 owns [512:1536)
— every core runs the identical program (true SPMD); causality and padding
are encoded in per-core additive mask inputs.

K/V projection is FULLY split across the pair: core h projects K^T and V
only for its own 1024 keys (global keys [h*1024:(h+1)*1024)), and the halves
are exchanged with four pair AllGathers (K keys 0:512-own, K keys 512:1024-own,
V ditto) through DRAM bounce buffers, pipelined on the CC stream so each
lands before first use.  Own-key projection output is staged in the low
half of the K^T / V SBUF tensors; the gather readback (both regions, so the
final key order is identical on both ranks) overwrites them with globally
ordered data.

On-chip layout is feature-major (all host-side transposes are free):
  xT/wT in, Q^T/K^T feature-major, V sequence-major.  Scores are computed
  as S^T[k, q] so softmax needs no on-chip transpose anywhere:
  exp via ACT (no max subtraction -- scaled scores are ~N(0,1), exp is
  safe in fp32), sum-of-exp via a ones-column matmul, AV accumulates
  out^T[e, q] with V as the stationary operand.  The per-query 1/sum is
  broadcast across partitions with a K=1 PE matmul (ones-column x sum-row)
  and applied by DVE during the PSUM->SBUF output copy.  Output returns
  as out^T and is transposed back on the host.  All matmuls bf16 with
  fp32 PSUM accumulation.

The kernel opens with a short burst of warm-up matmuls on a zeroed tile so
the PE HAM clock-gate reaches 8/8 (2.4 GHz) while the first input DMAs are
still in flight; the K projection runs d-outer so its first matmul only
needs ~0.4 MB of DMA.
"""

import functools

import ml_dtypes
import numpy as np

B = 4
S = 2048
D = 1024
P = 128
DCH = D // P            # 8 contraction chunks
QT = 512                # query-tile width
KO = 1024               # own keys per core (projection split)
CTX_A, CTX_B = 1024, 2048
NKA, NKB = CTX_A // P, CTX_B // P
NEG = np.float32(-30000.0)
WARMUP_MM = 14

_BF16 = ml_dtypes.bfloat16


@functools.lru_cache(maxsize=1)
def _build_nc():
    import concourse.bass as bass  # noqa: F401  (registers engines)
    import concourse.mybir as mybir
    from concourse import bacc, tile

    bf16 = mybir.dt.bfloat16
    f32 = mybir.dt.float32
    add = mybir.AluOpType.add
    mult = mybir.AluOpType.mult
    Exp = mybir.ActivationFunctionType.Exp
    PAIRS = [[2 * i, 2 * i + 1] for i in range(4)]

    nc = bacc.Bacc("TRN2", target_bir_lowering=False, debug=False, num_devices=8)

    xT = nc.declare_dram_parameter("xT", [D, KO], bf16, isOutput=False)
    xqT = nc.declare_dram_parameter("xqT", [D, 2 * QT], bf16, isOutput=False)
    wqT = nc.declare_dram_parameter("wqT", [D, D], bf16, isOutput=False)
    wkT = nc.declare_dram_parameter("wkT", [D, D], bf16, isOutput=False)
    wvT = nc.declare_dram_parameter("wvT", [D, D], bf16, isOutput=False)
    maskA = nc.declare_dram_parameter("maskA", [CTX_A, QT], f32, isOutput=False)
    maskB = nc.declare_dram_parameter("maskB", [CTX_B // 2, QT], f32,
                                      isOutput=False)
    outT = nc.declare_dram_parameter("outT", [D, 2 * QT], f32, isOutput=True)

    with tile.TileContext(nc) as tc:
        with (
            tc.tile_pool(name="kv", bufs=1) as kv,
            tc.tile_pool(name="dram", bufs=1, space="DRAM") as dram,
        ):
            # ---- persistent SBUF tensors --------------------------------
            kt_sb = [kv.tile([P, S], bf16, tag=f"kt{e}", name=f"kt{e}")
                     for e in range(DCH)]
            qt_sb = [kv.tile([P, 2 * QT], bf16, tag=f"qt{e}", name=f"qt{e}")
                     for e in range(DCH)]
            v_sb = [kv.tile([P, D], bf16, tag=f"v{t}", name=f"v{t}")
                    for t in range(S // P)]
            ones_sb = kv.tile([P, 1], bf16, tag="ones", name="ones")
            nc.gpsimd.memset(ones_sb[:], 1.0)
            onesr = kv.tile([1, P], f32, tag="onesr", name="onesr")
            nc.gpsimd.memset(onesr[:], 1.0)
            # touch the Exp LUT once so the lazy activation-table load isn't
            # on the first score tile's critical path
            scr = kv.tile([P, 1], f32, tag="scr", name="scr")
            nc.scalar.activation(scr[:], ones_sb[:], Exp)

            # DRAM bounce buffers for the pair K/V exchange (split in two
            # per tensor so the CC stream pipelines and early keys land
            # early).  Region r of each agout holds group-rank r's half.
            agin_k = [dram.tile([D, QT], bf16, name=f"agin_k{i}")
                      for i in range(2)]
            agout_k = [dram.tile([2 * D, QT], bf16, name=f"agout_k{i}")
                       for i in range(2)]
            agin_v = [dram.tile([4 * P, D], bf16, name=f"agin_v{i}")
                      for i in range(2)]
            agout_v = [dram.tile([8 * P, D], bf16, name=f"agout_v{i}")
                       for i in range(2)]

            # ---- phase 1: load inputs + QKV projections -----------------
            with (
                tc.tile_pool(name="inp", bufs=1) as inp,
                tc.tile_pool(name="pps", bufs=2, space="PSUM") as pps,
            ):
                x_sb = [inp.tile([P, KO], bf16, tag=f"x{d}", name=f"x{d}")
                        for d in range(DCH)]
                xq_sb = [inp.tile([P, 2 * QT], bf16, tag=f"xq{d}", name=f"xq{d}")
                         for d in range(DCH)]
                wq_sb = [inp.tile([P, D], bf16, tag=f"wq{d}", name=f"wq{d}")
                         for d in range(DCH)]
                wk_sb = [inp.tile([P, D], bf16, tag=f"wk{d}", name=f"wk{d}")
                         for d in range(DCH)]
                wv_sb = [inp.tile([P, D], bf16, tag=f"wv{d}", name=f"wv{d}")
                        for d in range(DCH)]
                warm = inp.tile([P, QT], bf16, tag="warm", name="warm")

                # PE warm-up: HAM un-throttles after ~3.4us of sustained
                # matmul activity; burn the initial DMA window on junk
                # matmuls so the real ones run at 2.4 GHz from the start.
                nc.gpsimd.memset(warm[:], 0.0)
                wps = pps.tile([P, QT], f32, tag="pj0", name="wps")
                for i in range(WARMUP_MM):
                    nc.tensor.matmul(wps[:], warm[:, 0:P], warm[:],
                                     start=True, stop=True,
                                     skip_group_check=True)

                # Load order = first-use order, issued on the SCALAR queue
                # (idle until phase 2) so the sync queue stays free for the
                # exchange DMAs, whose semaphore waits would otherwise
                # head-of-line-block these streaming loads.  K projection
                # runs d-outer, so interleave x (key-half 0) with wk per
                # d-chunk: the first matmul group only needs ~0.4 MB.
                for d in range(DCH):
                    rows = slice(d * P, (d + 1) * P)
                    nc.scalar.dma_start(out=x_sb[d][:, 0:QT],
                                        in_=xT[rows, 0:QT])
                    nc.scalar.dma_start(out=wk_sb[d][:], in_=wkT[rows, :])
                for d in range(DCH):
                    rows = slice(d * P, (d + 1) * P)
                    nc.scalar.dma_start(out=x_sb[d][:, QT:KO],
                                        in_=xT[rows, QT:KO])
                for d in range(DCH):
                    rows = slice(d * P, (d + 1) * P)
                    nc.scalar.dma_start(out=wv_sb[d][:], in_=wvT[rows, :])
                for d in range(DCH):
                    rows = slice(d * P, (d + 1) * P)
                    nc.scalar.dma_start(out=wq_sb[d][:], in_=wqT[rows, :])
                    nc.scalar.dma_start(out=xq_sb[d][:], in_=xqT[rows, :])

                # K^T for the 1024 own keys, half (512 keys) at a time so
                # each half's AllGather fires as early as possible.  Loop
                # d-outer with 4-wide e-groups accumulating in parallel
                # PSUM banks; the first group only waits on x[d0]/wk[d0].
                def k_proj_half(half):
                    hsl = slice(half * QT, (half + 1) * QT)
                    for eg in range(2):
                        es = range(eg * 4, eg * 4 + 4)
                        pss = [pps.tile([P, QT], f32, tag=f"pj{i}",
                                        name=f"kps{half}_{eg}_{i}")
                               for i in range(4)]
                        for d in range(DCH):
                            for i, e in enumerate(es):
                                nc.tensor.matmul(
                                    pss[i][:],
                                    wk_sb[d][:, e * P:(e + 1) * P],
                                    x_sb[d][:, hsl],
                                    start=(d == 0), stop=(d == DCH - 1),
                                    skip_group_check=True,
                                )
                        for i, e in enumerate(es):
                            nc.vector.tensor_copy(kt_sb[e][:, hsl], pss[i][:])
                    for e in range(DCH):
                        nc.sync.dma_start(out=agin_k[half][e * P:(e + 1) * P, :],
                                          in_=kt_sb[e][:, hsl])
                    nc.gpsimd.collective_compute(
                        "AllGather", mybir.AluOpType.bypass,
                        replica_groups=PAIRS,
                        ins=[agin_k[half][:]], outs=[agout_k[half][:]],
                    )

                # Read back BOTH regions of a gather: final key order is
                # global and identical on both ranks.  half h, region r ->
                # kt cols [r*1024 + h*512 : .. + 512).
                def k_readback(half):
                    for r in range(2):
                        csl = slice(r * KO + half * QT, r * KO + (half + 1) * QT)
                        for e in range(DCH):
                            nc.sync.dma_start(
                                out=kt_sb[e][:, csl],
                                in_=agout_k[half][r * D + e * P:
                                                  r * D + (e + 1) * P, :])

                def v_proj(t):
                    tsl = slice(t * P, (t + 1) * P)
                    pss = [pps.tile([P, QT], f32, tag=f"pj{eh}",
                                    name=f"vps{t}_{eh}") for eh in range(2)]
                    for d in range(DCH):
                        for eh in range(2):
                            nc.tensor.matmul(
                                pss[eh][:], x_sb[d][:, tsl],
                                wv_sb[d][:, eh * QT:(eh + 1) * QT],
                                start=(d == 0), stop=(d == DCH - 1),
                            )
                    for eh in range(2):
                        nc.vector.tensor_copy(
                            v_sb[t][:, eh * QT:(eh + 1) * QT], pss[eh][:])

                def v_gather_half(half):
                    for t in range(half * 4, half * 4 + 4):
                        nc.sync.dma_start(
                            out=agin_v[half][(t - half * 4) * P:
                                             (t - half * 4 + 1) * P, :],
                            in_=v_sb[t][:])
                    nc.gpsimd.collective_compute(
                        "AllGather", mybir.AluOpType.bypass,
                        replica_groups=PAIRS,
                        ins=[agin_v[half][:]], outs=[agout_v[half][:]],
                    )

                # readback: half h region r -> v_sb[r*8 + h*4 + i]
                def v_readback(half):
                    for r in range(2):
                        for i in range(4):
                            nc.sync.dma_start(
                                out=v_sb[r * 8 + half * 4 + i][:],
                                in_=agout_v[half][(r * 4 + i) * P:
                                                  (r * 4 + i + 1) * P, :])

                # Emission order staggers the sync-queue DMAs so each one's
                # semaphore wait resolves roughly when the queue reaches it
                # (a blocked DMA stalls everything behind it on its queue).
                k_proj_half(0)
                k_proj_half(1)
                k_readback(0)
                for t in range(4):
                    v_proj(t)
                v_gather_half(0)
                k_readback(1)
                for t in range(4, 8):
                    v_proj(t)
                v_gather_half(1)
                v_readback(0)

                # Q^T[e, q]: one wq weight tile drives both query halves.
                for e in range(DCH):
                    esl = slice(e * P, (e + 1) * P)
                    pss = [pps.tile([P, QT], f32, tag=f"pj{qh}",
                                    name=f"qps{e}_{qh}") for qh in range(2)]
                    for d in range(DCH):
                        for qh in range(2):
                            nc.tensor.matmul(
                                pss[qh][:], wq_sb[d][:, esl],
                                xq_sb[d][:, qh * QT:(qh + 1) * QT],
                                start=(d == 0), stop=(d == DCH - 1),
                            )
                    for qh in range(2):
                        nc.vector.tensor_copy(
                            qt_sb[e][:, qh * QT:(qh + 1) * QT], pss[qh][:])
                v_readback(1)

            # ---- phase 2: attention, one 512-query tile at a time -------
            with (
                tc.tile_pool(name="pp", bufs=2) as pp,
                tc.tile_pool(name="mk", bufs=4) as mkp,
                tc.tile_pool(name="ost", bufs=4) as ost,
                tc.tile_pool(name="msc", bufs=2) as msc,
                tc.tile_pool(name="scp", bufs=4, space="PSUM") as scp,
                tc.tile_pool(name="aps", bufs=2, space="PSUM") as aps,
            ):
                qslA, qslB = slice(0, QT), slice(QT, 2 * QT)

                def exp_tile(t_idx, k, ps):
                    ph = pp.tile([P, QT], bf16, tag=f"p{k}",
                                 name=f"ph{t_idx}_{k}")
                    # exp((score + mask) / sqrt(1024)); no max subtraction
                    nc.scalar.activation(ph[:], ps[:], Exp, scale=0.03125)
                    return ph

                def recip_chain(t_idx, sps):
                    srow = msc.tile([1, QT], f32, tag="srow", name=f"srow{t_idx}")
                    nc.vector.tensor_copy(srow[:], sps[:])
                    bc = aps.tile([P, QT], f32, tag="sum", name=f"bc{t_idx}")
                    nc.tensor.matmul(bc[:], onesr[:, 0:P], srow[:],
                                     start=True, stop=True)
                    recb = msc.tile([P, QT], f32, tag="recb", name=f"recb{t_idx}")
                    nc.vector.reciprocal_approx_fast(out=recb[:], in_=bc[:])
                    return recb

                # Key chunks 0..7 are shared by both query tiles: interleave
                # their score matmuls so each KT slice is loaded into the PE
                # array once and drives two matmuls.  Tile B is fully valid
                # on chunks 0..7 (no mask); tile A is masked everywhere.
                phatA, phatB = [], []
                for k in range(NKA):
                    ksl = slice(k * P, (k + 1) * P)
                    mk = mkp.tile([P, QT], f32, tag="mask0", name=f"m0_{k}")
                    nc.sync.dma_start(out=mk[:], in_=maskA[k * P:(k + 1) * P, :])
                    psA = scp.tile([P, QT], f32, tag="sc", name=f"scA{k}")
                    psB = scp.tile([P, QT], f32, tag="sc", name=f"scB{k}")
                    for e in range(DCH):
                        nc.tensor.matmul(
                            psA[:], kt_sb[e][:, ksl], qt_sb[e][:, qslA],
                            start=(e == 0), stop=(e == DCH - 1),
                            skip_group_check=True,
                        )
                        nc.tensor.matmul(
                            psB[:], kt_sb[e][:, ksl], qt_sb[e][:, qslB],
                            start=(e == 0), stop=(e == DCH - 1),
                            skip_group_check=True,
                        )
                    nc.vector.tensor_tensor(psA[:], psA[:], mk[:], op=add)
                    phatA.append(exp_tile(0, k, psA))
                    phatB.append(exp_tile(1, k, psB))

                # tile A's softmax denominator + 1/sum while tile B's upper
                # chunks are still streaming on PE
                spsA = aps.tile([1, QT], f32, tag="sum", name="sumA")
                for k in range(NKA):
                    nc.tensor.matmul(
                        spsA[:], ones_sb[:, 0:1], phatA[k][:],
                        start=(k == 0), stop=(k == NKA - 1),
                    )
                recbA = recip_chain(0, spsA)

                # tile B's private upper-half chunks (the only maskable ones)
                for k in range(NKA, NKB):
                    ksl = slice(k * P, (k + 1) * P)
                    mk = mkp.tile([P, QT], f32, tag="mask1", name=f"m1_{k}")
                    mrow = (k - NKA) * P
                    nc.sync.dma_start(out=mk[:], in_=maskB[mrow:mrow + P, :])
                    psB = scp.tile([P, QT], f32, tag="sc", name=f"scB{k}")
                    for e in range(DCH):
                        nc.tensor.matmul(
                            psB[:], kt_sb[e][:, ksl], qt_sb[e][:, qslB],
                            start=(e == 0), stop=(e == DCH - 1),
                        )
                    nc.vector.tensor_tensor(psB[:], psB[:], mk[:], op=add)
                    phatB.append(exp_tile(1, k, psB))

                spsB = aps.tile([1, QT], f32, tag="sum", name="sumB")
                for k in range(NKB):
                    nc.tensor.matmul(
                        spsB[:], ones_sb[:, 0:1], phatB[k][:],
                        start=(k == 0), stop=(k == NKB - 1),
                    )
                recbB = recip_chain(1, spsB)

                # AV, interleaved the same way: one V slice load drives both
                # tiles' accumulations for chunks 0..7.
                for e in range(DCH):
                    esl = slice(e * P, (e + 1) * P)
                    psA = aps.tile([P, QT], f32, tag="av", name=f"avA{e}")
                    psB = aps.tile([P, QT], f32, tag="av", name=f"avB{e}")
                    for k in range(NKB):
                        if k < NKA:
                            nc.tensor.matmul(
                                psA[:], v_sb[k][:, esl], phatA[k][:],
                                start=(k == 0), stop=(k == NKA - 1),
                                skip_group_check=True,
                            )
                        nc.tensor.matmul(
                            psB[:], v_sb[k][:, esl], phatB[k][:],
                            start=(k == 0), stop=(k == NKB - 1),
                            skip_group_check=True,
                        )
                    for t_idx, ps, recb, qsl in ((0, psA, recbA, qslA),
                                                 (1, psB, recbB, qslB)):
                        ot = ost.tile([P, QT], f32, tag="ot",
                                      name=f"ot{t_idx}_{e}")
                        nc.vector.tensor_tensor(ot[:], ps[:], recb[:], op=mult)
                        nc.sync.dma_start(out=outT[esl, qsl], in_=ot[:])

    nc.compile()
    return nc


# h=0 -> query blocks [0:512) (tile A) and [1536:2048) (tile B)
# h=1 -> query blocks [512:1024) (tile A) and [1024:1536) (tile B)
_QSTARTS = ((0, 3 * QT), (QT, 2 * QT))


def _make_mask(q0: int, ctx: int) -> np.ndarray:
    k = np.arange(ctx)[:, None]
    q = q0 + np.arange(QT)[None, :]
    return np.where(k <= q, np.float32(0.0), NEG).astype(np.float32)


def _in_maps(x, Wk, Wq, Wv):
    wq_t = np.ascontiguousarray(Wk.T.astype(_BF16))   # ref swap: q uses Wk
    wk_t = np.ascontiguousarray(Wq.T.astype(_BF16))
    wv_t = np.ascontiguousarray(Wv.T.astype(_BF16))
    maps = []
    for c in range(8):
        b, h = divmod(c, 2)
        qa, qb = _QSTARTS[h]
        xb = x[b].astype(_BF16)
        # own keys for the K/V projection split
        x_t = np.ascontiguousarray(xb[h * KO:(h + 1) * KO].T)
        xq_t = np.ascontiguousarray(
            np.concatenate([xb[qa:qa + QT], xb[qb:qb + QT]], axis=0).T
        )
        maps.append({
            "xT": x_t,
            "xqT": xq_t,
            "wqT": wq_t,
            "wkT": wk_t,
            "wvT": wv_t,
            "maskA": _make_mask(qa, CTX_A),
            # tile B chunks 0..7 are fully valid on every core; only the
            # upper half of its context can need masking
            "maskB": _make_mask(qb, CTX_B)[CTX_B // 2:],
        })
    return maps


def _assemble(results):
    out = np.empty((B, S, D), dtype=np.float32)
    for c, res in enumerate(results):
        b, h = divmod(c, 2)
        qa, qb = _QSTARTS[h]
        o = res["outT"]
        out[b, qa:qa + QT] = o[:, 0:QT].T
        out[b, qb:qb + QT] = o[:, QT:2 * QT].T
    return out


def kernel(x, Wk, Wq, Wv, _trace=False):
    from concourse.bass_utils import run_bass_kernel_spmd

    nc = _build_nc()
    res = run_bass_kernel_spmd(nc, _in_maps(x, Wk, Wq, Wv), list(range(8)),
                               trace=_trace)
    out = _assemble(res.results)
    if _trace:
        return out, res
    return out


# revision 9
# speedup vs baseline: 1.2595x; 1.2595x over previous
"""Causal single-head attention on 8 TRN2 NeuronCores.

Problem (hardcoded): x [4, 2048, 1024] f32; Wk, Wq, Wv [1024, 1024] f32.
  q = x @ Wk.T ; k = x @ Wq.T ; v = x @ Wv.T        (note ref's q/k weight swap)
  out = softmax(mask(q @ k.T) / sqrt(1024)) @ v

Sharding: 2 cores per batch. Core h of a batch computes 1024 queries as two
512-query tiles: tile A with a 1024-key context, tile B with a 2048-key
context.  h=0 owns query blocks [0:512) + [1536:2048), h=1 owns [512:1536)
— every core runs the identical program (true SPMD); causality and padding
are encoded in per-core additive mask inputs.

K/V projection is FULLY split across the pair: core h projects K^T and V
only for its own 1024 keys (global keys [h*1024:(h+1)*1024)), and the halves
are exchanged with four pair AllGathers (K keys 0:512-own, K keys 512:1024-own,
V ditto) through DRAM bounce buffers, pipelined on the CC stream so each
lands before first use.  Own-key projection output is staged in the low
half of the K^T / V SBUF tensors; the gather readback (both regions, so the
final key order is identical on both ranks) overwrites them with globally
ordered data.

On-chip layout is feature-major (all host-side transposes are free):
  xT/wT in, Q^T/K^T feature-major, V sequence-major.  Scores are computed
  as S^T[k, q] so softmax needs no on-chip transpose anywhere:
  exp via ACT (no max subtraction -- scaled scores are ~N(0,1), exp is
  safe in fp32), sum-of-exp via a ones-column matmul, AV accumulates
  out^T[e, q] with V as the stationary operand.  The per-query 1/sum is
  broadcast across partitions with a K=1 PE matmul (ones-column x sum-row)
  and applied by DVE during the PSUM->SBUF output copy.  Output returns
  as out^T and is transposed back on the host.  All matmuls bf16 with
  fp32 PSUM accumulation.

The kernel opens with a short burst of warm-up matmuls on a zeroed tile so
the PE HAM clock-gate reaches 8/8 (2.4 GHz) while the first input DMAs are
still in flight; the K projection runs d-outer so its first matmul only
needs ~0.4 MB of DMA.
"""

import functools

import ml_dtypes
import numpy as np

B = 4
S = 2048
D = 1024
P = 128
DCH = D // P            # 8 contraction chunks
QT = 512                # query-tile width
KO = 1024               # own keys per core (projection split)
CTX_A, CTX_B = 1024, 2048
NKA, NKB = CTX_A // P, CTX_B // P
NEG = np.float32(-30000.0)
WARMUP_MM = 14

_BF16 = ml_dtypes.bfloat16


@functools.lru_cache(maxsize=1)
def _build_nc():
    import concourse.bass as bass  # noqa: F401  (registers engines)
    import concourse.mybir as mybir
    from concourse import bacc, tile

    bf16 = mybir.dt.bfloat16
    f32 = mybir.dt.float32
    add = mybir.AluOpType.add
    mult = mybir.AluOpType.mult
    Exp = mybir.ActivationFunctionType.Exp
    PAIRS = [[2 * i, 2 * i + 1] for i in range(4)]

    nc = bacc.Bacc("TRN2", target_bir_lowering=False, debug=False, num_devices=8)

    xT = nc.declare_dram_parameter("xT", [D, KO], bf16, isOutput=False)
    xqT = nc.declare_dram_parameter("xqT", [D, 2 * QT], bf16, isOutput=False)
    wqT = nc.declare_dram_parameter("wqT", [D, D], bf16, isOutput=False)
    wkT = nc.declare_dram_parameter("wkT", [D, D], bf16, isOutput=False)
    wvT = nc.declare_dram_parameter("wvT", [D, D], bf16, isOutput=False)
    maskA = nc.declare_dram_parameter("maskA", [CTX_A, QT], bf16, isOutput=False)
    maskB = nc.declare_dram_parameter("maskB", [CTX_B // 2, QT], bf16,
                                      isOutput=False)
    outT = nc.declare_dram_parameter("outT", [D, 2 * QT], f32, isOutput=True)

    with tile.TileContext(nc) as tc:
        with (
            tc.tile_pool(name="kv", bufs=1) as kv,
            tc.tile_pool(name="dram", bufs=1, space="DRAM") as dram,
        ):
            # ---- persistent SBUF tensors --------------------------------
            kt_sb = [kv.tile([P, S], bf16, tag=f"kt{e}", name=f"kt{e}")
                     for e in range(DCH)]
            qt_sb = [kv.tile([P, 2 * QT], bf16, tag=f"qt{e}", name=f"qt{e}")
                     for e in range(DCH)]
            v_sb = [kv.tile([P, D], bf16, tag=f"v{t}", name=f"v{t}")
                    for t in range(S // P)]
            ones_sb = kv.tile([P, 1], bf16, tag="ones", name="ones")
            nc.gpsimd.memset(ones_sb[:], 1.0)
            onesr = kv.tile([1, P], f32, tag="onesr", name="onesr")
            nc.gpsimd.memset(onesr[:], 1.0)
            # touch the Exp LUT once so the lazy activation-table load isn't
            # on the first score tile's critical path
            scr = kv.tile([P, 1], f32, tag="scr", name="scr")
            nc.scalar.activation(scr[:], ones_sb[:], Exp)
            mA_sb = [kv.tile([P, QT], bf16, tag=f"mA{k}", name=f"mA{k}")
                     for k in range(NKA)]
            mB_sb = [kv.tile([P, QT], bf16, tag=f"mB{k}", name=f"mB{k}")
                     for k in range(NKB - NKA)]

            # DRAM bounce buffers for the pair K/V exchange (split in two
            # per tensor so the CC stream pipelines and early keys land
            # early).  Region r of each agout holds group-rank r's half.
            agin_k = [dram.tile([D, QT], bf16, name=f"agin_k{i}")
                      for i in range(2)]
            agout_k = [dram.tile([2 * D, QT], bf16, name=f"agout_k{i}")
                       for i in range(2)]
            agin_v = [dram.tile([4 * P, D], bf16, name=f"agin_v{i}")
                      for i in range(2)]
            agout_v = [dram.tile([8 * P, D], bf16, name=f"agout_v{i}")
                       for i in range(2)]

            # ---- phase 1: load inputs + QKV projections -----------------
            with (
                tc.tile_pool(name="inp", bufs=1) as inp,
                tc.tile_pool(name="pps", bufs=2, space="PSUM") as pps,
            ):
                x_sb = [inp.tile([P, KO], bf16, tag=f"x{d}", name=f"x{d}")
                        for d in range(DCH)]
                xq_sb = [inp.tile([P, 2 * QT], bf16, tag=f"xq{d}", name=f"xq{d}")
                         for d in range(DCH)]
                wq_sb = [inp.tile([P, D], bf16, tag=f"wq{d}", name=f"wq{d}")
                         for d in range(DCH)]
                wk_sb = [inp.tile([P, D], bf16, tag=f"wk{d}", name=f"wk{d}")
                         for d in range(DCH)]
                wv_sb = [inp.tile([P, D], bf16, tag=f"wv{d}", name=f"wv{d}")
                        for d in range(DCH)]
                warm = inp.tile([P, QT], bf16, tag="warm", name="warm")

                # PE warm-up: HAM un-throttles after ~3.4us of sustained
                # matmul activity; burn the initial DMA window on junk
                # matmuls so the real ones run at 2.4 GHz from the start.
                nc.gpsimd.memset(warm[:], 0.0)
                wps = pps.tile([P, QT], f32, tag="pj0", name="wps")
                for i in range(WARMUP_MM):
                    nc.tensor.matmul(wps[:], warm[:, 0:P], warm[:],
                                     start=True, stop=True,
                                     skip_group_check=True)

                # Load order = first-use order on the sync queue (the
                # fast DMA issuer; engine-issued DMAs block their queue
                # until the transfer completes).  K projection runs
                # d-outer, so interleave x (key-half 0) with wk per
                # d-chunk: the first matmul group only needs ~0.4 MB.
                for d in range(DCH):
                    rows = slice(d * P, (d + 1) * P)
                    nc.sync.dma_start(out=x_sb[d][:, 0:QT],
                                      in_=xT[rows, 0:QT])
                    nc.sync.dma_start(out=wk_sb[d][:], in_=wkT[rows, :])
                for d in range(DCH):
                    rows = slice(d * P, (d + 1) * P)
                    nc.sync.dma_start(out=x_sb[d][:, QT:KO],
                                      in_=xT[rows, QT:KO])
                for d in range(DCH):
                    rows = slice(d * P, (d + 1) * P)
                    nc.sync.dma_start(out=wv_sb[d][:], in_=wvT[rows, :])

                # K^T for the 1024 own keys, half (512 keys) at a time so
                # each half's AllGather fires as early as possible.  Loop
                # d-outer with 4-wide e-groups accumulating in parallel
                # PSUM banks; the first group only waits on x[d0]/wk[d0].
                def k_proj_half(half):
                    hsl = slice(half * QT, (half + 1) * QT)
                    for eg in range(2):
                        es = range(eg * 4, eg * 4 + 4)
                        pss = [pps.tile([P, QT], f32, tag=f"pj{i}",
                                        name=f"kps{half}_{eg}_{i}")
                               for i in range(4)]
                        for d in range(DCH):
                            for i, e in enumerate(es):
                                nc.tensor.matmul(
                                    pss[i][:],
                                    wk_sb[d][:, e * P:(e + 1) * P],
                                    x_sb[d][:, hsl],
                                    start=(d == 0), stop=(d == DCH - 1),
                                    skip_group_check=True,
                                )
                        for i, e in enumerate(es):
                            nc.vector.tensor_copy(kt_sb[e][:, hsl], pss[i][:])
                    for e in range(DCH):
                        nc.sync.dma_start(out=agin_k[half][e * P:(e + 1) * P, :],
                                          in_=kt_sb[e][:, hsl])
                    nc.gpsimd.collective_compute(
                        "AllGather", mybir.AluOpType.bypass,
                        replica_groups=PAIRS,
                        ins=[agin_k[half][:]], outs=[agout_k[half][:]],
                    )

                # Read back BOTH regions of a gather: final key order is
                # global and identical on both ranks.  half h, region r ->
                # kt cols [r*1024 + h*512 : .. + 512).
                def k_readback(half):
                    for r in range(2):
                        csl = slice(r * KO + half * QT, r * KO + (half + 1) * QT)
                        for e in range(DCH):
                            nc.sync.dma_start(
                                out=kt_sb[e][:, csl],
                                in_=agout_k[half][r * D + e * P:
                                                  r * D + (e + 1) * P, :])

                def v_proj(t):
                    tsl = slice(t * P, (t + 1) * P)
                    pss = [pps.tile([P, QT], f32, tag=f"pj{eh}",
                                    name=f"vps{t}_{eh}") for eh in range(2)]
                    for d in range(DCH):
                        for eh in range(2):
                            nc.tensor.matmul(
                                pss[eh][:], x_sb[d][:, tsl],
                                wv_sb[d][:, eh * QT:(eh + 1) * QT],
                                start=(d == 0), stop=(d == DCH - 1),
                            )
                    for eh in range(2):
                        nc.vector.tensor_copy(
                            v_sb[t][:, eh * QT:(eh + 1) * QT], pss[eh][:])

                def v_gather_half(half):
                    for t in range(half * 4, half * 4 + 4):
                        nc.sync.dma_start(
                            out=agin_v[half][(t - half * 4) * P:
                                             (t - half * 4 + 1) * P, :],
                            in_=v_sb[t][:])
                    nc.gpsimd.collective_compute(
                        "AllGather", mybir.AluOpType.bypass,
                        replica_groups=PAIRS,
                        ins=[agin_v[half][:]], outs=[agout_v[half][:]],
                    )

                # readback: half h region r -> v_sb[r*8 + h*4 + i]
                def v_readback(half):
                    for r in range(2):
                        for i in range(4):
                            nc.sync.dma_start(
                                out=v_sb[r * 8 + half * 4 + i][:],
                                in_=agout_v[half][(r * 4 + i) * P:
                                                  (r * 4 + i + 1) * P, :])

                # Emission order staggers the sync-queue DMAs so each one's
                # semaphore wait resolves roughly when the queue reaches it
                # (a blocked DMA stalls everything behind it on its queue).
                k_proj_half(0)
                # remaining inputs enter the sync queue behind agin_k0 (so
                # they don't delay the first gather): wq/xq for Q proj
                # (needed ~60us in) and the persistent mask tiles (needed
                # at the first score chunk's DVE mask-add).
                for d in range(DCH):
                    rows = slice(d * P, (d + 1) * P)
                    nc.sync.dma_start(out=wq_sb[d][:], in_=wqT[rows, :])
                    nc.sync.dma_start(out=xq_sb[d][:], in_=xqT[rows, :])
                for k in range(NKA):
                    nc.sync.dma_start(out=mA_sb[k][:],
                                      in_=maskA[k * P:(k + 1) * P, :])
                for k in range(NKB - NKA):
                    nc.sync.dma_start(out=mB_sb[k][:],
                                      in_=maskB[k * P:(k + 1) * P, :])
                k_proj_half(1)
                k_readback(0)
                for t in range(4):
                    v_proj(t)
                v_gather_half(0)
                k_readback(1)
                for t in range(4, 8):
                    v_proj(t)
                v_gather_half(1)
                v_readback(0)

                # Q^T[e, q]: one wq weight tile drives both query halves.
                for e in range(DCH):
                    esl = slice(e * P, (e + 1) * P)
                    pss = [pps.tile([P, QT], f32, tag=f"pj{qh}",
                                    name=f"qps{e}_{qh}") for qh in range(2)]
                    for d in range(DCH):
                        for qh in range(2):
                            nc.tensor.matmul(
                                pss[qh][:], wq_sb[d][:, esl],
                                xq_sb[d][:, qh * QT:(qh + 1) * QT],
                                start=(d == 0), stop=(d == DCH - 1),
                            )
                    for qh in range(2):
                        nc.vector.tensor_copy(
                            qt_sb[e][:, qh * QT:(qh + 1) * QT], pss[qh][:])
                v_readback(1)

            # ---- phase 2: attention, one 512-query tile at a time -------
            with (
                tc.tile_pool(name="pp", bufs=2) as pp,
                tc.tile_pool(name="ost", bufs=4) as ost,
                tc.tile_pool(name="msc", bufs=2) as msc,
                tc.tile_pool(name="scp", bufs=4, space="PSUM") as scp,
                tc.tile_pool(name="aps", bufs=2, space="PSUM") as aps,
            ):
                qslA, qslB = slice(0, QT), slice(QT, 2 * QT)

                def exp_tile(t_idx, k, ps):
                    ph = pp.tile([P, QT], bf16, tag=f"p{k}",
                                 name=f"ph{t_idx}_{k}")
                    # exp((score + mask) / sqrt(1024)); no max subtraction
                    nc.scalar.activation(ph[:], ps[:], Exp, scale=0.03125)
                    return ph

                def recip_chain(t_idx, sps):
                    srow = msc.tile([1, QT], f32, tag="srow", name=f"srow{t_idx}")
                    nc.vector.tensor_copy(srow[:], sps[:])
                    bc = aps.tile([P, QT], f32, tag="sum", name=f"bc{t_idx}")
                    nc.tensor.matmul(bc[:], onesr[:, 0:P], srow[:],
                                     start=True, stop=True)
                    recb = msc.tile([P, QT], f32, tag="recb", name=f"recb{t_idx}")
                    nc.vector.reciprocal_approx_fast(out=recb[:], in_=bc[:])
                    return recb

                # Key chunks 0..7 are shared by both query tiles: interleave
                # their score matmuls so each KT slice is loaded into the PE
                # array once and drives two matmuls.  Tile B is fully valid
                # on chunks 0..7 (no mask); tile A is masked everywhere.
                phatA, phatB = [], []
                for k in range(NKA):
                    ksl = slice(k * P, (k + 1) * P)
                    psA = scp.tile([P, QT], f32, tag="sc", name=f"scA{k}")
                    psB = scp.tile([P, QT], f32, tag="sc", name=f"scB{k}")
                    for e in range(DCH):
                        nc.tensor.matmul(
                            psA[:], kt_sb[e][:, ksl], qt_sb[e][:, qslA],
                            start=(e == 0), stop=(e == DCH - 1),
                            skip_group_check=True,
                        )
                        nc.tensor.matmul(
                            psB[:], kt_sb[e][:, ksl], qt_sb[e][:, qslB],
                            start=(e == 0), stop=(e == DCH - 1),
                            skip_group_check=True,
                        )
                    nc.vector.tensor_tensor(psA[:], psA[:], mA_sb[k][:], op=add)
                    phatA.append(exp_tile(0, k, psA))
                    phatB.append(exp_tile(1, k, psB))

                # tile A's softmax denominator + 1/sum while tile B's upper
                # chunks are still streaming on PE
                spsA = aps.tile([1, QT], f32, tag="sum", name="sumA")
                for k in range(NKA):
                    nc.tensor.matmul(
                        spsA[:], ones_sb[:, 0:1], phatA[k][:],
                        start=(k == 0), stop=(k == NKA - 1),
                    )
                recbA = recip_chain(0, spsA)

                # tile B's private upper-half chunks (the only maskable ones)
                for k in range(NKA, NKB):
                    ksl = slice(k * P, (k + 1) * P)
                    psB = scp.tile([P, QT], f32, tag="sc", name=f"scB{k}")
                    for e in range(DCH):
                        nc.tensor.matmul(
                            psB[:], kt_sb[e][:, ksl], qt_sb[e][:, qslB],
                            start=(e == 0), stop=(e == DCH - 1),
                        )
                    nc.vector.tensor_tensor(psB[:], psB[:], mB_sb[k - NKA][:],
                                            op=add)
                    phatB.append(exp_tile(1, k, psB))

                spsB = aps.tile([1, QT], f32, tag="sum", name="sumB")
                for k in range(NKB):
                    nc.tensor.matmul(
                        spsB[:], ones_sb[:, 0:1], phatB[k][:],
                        start=(k == 0), stop=(k == NKB - 1),
                    )
                recbB = recip_chain(1, spsB)

                # AV, interleaved the same way: one V slice load drives both
                # tiles' accumulations for chunks 0..7.
                for e in range(DCH):
                    esl = slice(e * P, (e + 1) * P)
                    psA = aps.tile([P, QT], f32, tag="av", name=f"avA{e}")
                    psB = aps.tile([P, QT], f32, tag="av", name=f"avB{e}")
                    for k in range(NKB):
                        if k < NKA:
                            nc.tensor.matmul(
                                psA[:], v_sb[k][:, esl], phatA[k][:],
                                start=(k == 0), stop=(k == NKA - 1),
                                skip_group_check=True,
                            )
                        nc.tensor.matmul(
                            psB[:], v_sb[k][:, esl], phatB[k][:],
                            start=(k == 0), stop=(k == NKB - 1),
                            skip_group_check=True,
                        )
                    for t_idx, ps, recb, qsl in ((0, psA, recbA, qslA),
                                                 (1, psB, recbB, qslB)):
                        ot = ost.tile([P, QT], f32, tag="ot",
                                      name=f"ot{t_idx}_{e}")
                        nc.vector.tensor_tensor(ot[:], ps[:], recb[:], op=mult)
                        nc.sync.dma_start(out=outT[esl, qsl], in_=ot[:])

    nc.compile()
    return nc


# h=0 -> query blocks [0:512) (tile A) and [1536:2048) (tile B)
# h=1 -> query blocks [512:1024) (tile A) and [1024:1536) (tile B)
_QSTARTS = ((0, 3 * QT), (QT, 2 * QT))


def _make_mask(q0: int, ctx: int) -> np.ndarray:
    k = np.arange(ctx)[:, None]
    q = q0 + np.arange(QT)[None, :]
    return np.where(k <= q, np.float32(0.0), NEG).astype(_BF16)


def _in_maps(x, Wk, Wq, Wv):
    wq_t = np.ascontiguousarray(Wk.T.astype(_BF16))   # ref swap: q uses Wk
    wk_t = np.ascontiguousarray(Wq.T.astype(_BF16))
    wv_t = np.ascontiguousarray(Wv.T.astype(_BF16))
    maps = []
    for c in range(8):
        b, h = divmod(c, 2)
        qa, qb = _QSTARTS[h]
        xb = x[b].astype(_BF16)
        # own keys for the K/V projection split
        x_t = np.ascontiguousarray(xb[h * KO:(h + 1) * KO].T)
        xq_t = np.ascontiguousarray(
            np.concatenate([xb[qa:qa + QT], xb[qb:qb + QT]], axis=0).T
        )
        maps.append({
            "xT": x_t,
            "xqT": xq_t,
            "wqT": wq_t,
            "wkT": wk_t,
            "wvT": wv_t,
            "maskA": _make_mask(qa, CTX_A),
            # tile B chunks 0..7 are fully valid on every core; only the
            # upper half of its context can need masking
            "maskB": _make_mask(qb, CTX_B)[CTX_B // 2:],
        })
    return maps


def _assemble(results):
    out = np.empty((B, S, D), dtype=np.float32)
    for c, res in enumerate(results):
        b, h = divmod(c, 2)
        qa, qb = _QSTARTS[h]
        o = res["outT"]
        out[b, qa:qa + QT] = o[:, 0:QT].T
        out[b, qb:qb + QT] = o[:, QT:2 * QT].T
    return out


def kernel(x, Wk, Wq, Wv, _trace=False):
    from concourse.bass_utils import run_bass_kernel_spmd

    nc = _build_nc()
    res = run_bass_kernel_spmd(nc, _in_maps(x, Wk, Wq, Wv), list(range(8)),
                               trace=_trace)
    out = _assemble(res.results)
    if _trace:
        return out, res
    return out


# revision 11
# speedup vs baseline: 1.2802x; 1.0165x over previous
"""Causal single-head attention on 8 TRN2 NeuronCores.

Problem (hardcoded): x [4, 2048, 1024] f32; Wk, Wq, Wv [1024, 1024] f32.
  q = x @ Wk.T ; k = x @ Wq.T ; v = x @ Wv.T        (note ref's q/k weight swap)
  out = softmax(mask(q @ k.T) / sqrt(1024)) @ v

Sharding: 2 cores per batch. Core h of a batch computes 1024 queries as two
512-query tiles: tile A with a 1024-key context, tile B with a 2048-key
context.  h=0 owns query blocks [0:512) + [1536:2048), h=1 owns [512:1536)
— every core runs the identical program (true SPMD); causality and padding
are encoded in per-core additive mask inputs.

K/V projection is FULLY split across the pair: core h projects K^T and V
only for its own 1024 keys (global keys [h*1024:(h+1)*1024)), and the halves
are exchanged with four pair AllGathers (K keys 0:512-own, K keys 512:1024-own,
V ditto) through DRAM bounce buffers, pipelined on the CC stream so each
lands before first use.  Own-key projection output is staged in the low
half of the K^T / V SBUF tensors; the gather readback (both regions, so the
final key order is identical on both ranks) overwrites them with globally
ordered data.

On-chip layout is feature-major (all host-side transposes are free):
  xT/wT in, Q^T/K^T feature-major, V sequence-major.  Scores are computed
  as S^T[k, q] so softmax needs no on-chip transpose anywhere:
  exp via ACT (no max subtraction -- scaled scores are ~N(0,1), exp is
  safe in fp32), sum-of-exp via a ones-column matmul, AV accumulates
  out^T[e, q] with V as the stationary operand.  The per-query 1/sum is
  broadcast across partitions with a K=1 PE matmul (ones-column x sum-row)
  and applied by DVE during the PSUM->SBUF output copy.  Output returns
  as out^T and is transposed back on the host.  All matmuls bf16 with
  fp32 PSUM accumulation.

The kernel opens with a short burst of warm-up matmuls on a zeroed tile so
the PE HAM clock-gate reaches 8/8 (2.4 GHz) while the first input DMAs are
still in flight; the K projection runs d-outer so its first matmul only
needs ~0.4 MB of DMA.
"""

import functools

import ml_dtypes
import numpy as np

B = 4
S = 2048
D = 1024
P = 128
DCH = D // P            # 8 contraction chunks
QT = 512                # query-tile width
KO = 1024               # own keys per core (projection split)
CTX_A, CTX_B = 1024, 2048
NKA, NKB = CTX_A // P, CTX_B // P
NEG = np.float32(-30000.0)
WARMUP_MM = 18

_BF16 = ml_dtypes.bfloat16


@functools.lru_cache(maxsize=1)
def _build_nc():
    import concourse.bass as bass  # noqa: F401  (registers engines)
    import concourse.mybir as mybir
    from concourse import bacc, tile

    bf16 = mybir.dt.bfloat16
    f32 = mybir.dt.float32
    add = mybir.AluOpType.add
    mult = mybir.AluOpType.mult
    Exp = mybir.ActivationFunctionType.Exp
    PAIRS = [[2 * i, 2 * i + 1] for i in range(4)]

    nc = bacc.Bacc("TRN2", target_bir_lowering=False, debug=False, num_devices=8)

    xT = nc.declare_dram_parameter("xT", [D, KO], bf16, isOutput=False)
    xqT = nc.declare_dram_parameter("xqT", [D, 2 * QT], bf16, isOutput=False)
    wqT = nc.declare_dram_parameter("wqT", [D, D], bf16, isOutput=False)
    wkT = nc.declare_dram_parameter("wkT", [D, D], bf16, isOutput=False)
    wvT = nc.declare_dram_parameter("wvT", [D, D], bf16, isOutput=False)
    maskA = nc.declare_dram_parameter("maskA", [CTX_A, QT], bf16, isOutput=False)
    maskB = nc.declare_dram_parameter("maskB", [CTX_B // 2, QT], bf16,
                                      isOutput=False)
    outT = nc.declare_dram_parameter("outT", [D, 2 * QT], f32, isOutput=True)

    with tile.TileContext(nc) as tc:
        with (
            tc.tile_pool(name="kv", bufs=1) as kv,
            tc.tile_pool(name="dram", bufs=1, space="DRAM") as dram,
        ):
            # ---- persistent SBUF tensors --------------------------------
            kt_sb = [kv.tile([P, S], bf16, tag=f"kt{e}", name=f"kt{e}")
                     for e in range(DCH)]
            qt_sb = [kv.tile([P, 2 * QT], bf16, tag=f"qt{e}", name=f"qt{e}")
                     for e in range(DCH)]
            v_sb = [kv.tile([P, D], bf16, tag=f"v{t}", name=f"v{t}")
                    for t in range(S // P)]
            ones_sb = kv.tile([P, 1], bf16, tag="ones", name="ones")
            nc.gpsimd.memset(ones_sb[:], 1.0)
            onesr = kv.tile([1, P], f32, tag="onesr", name="onesr")
            nc.gpsimd.memset(onesr[:], 1.0)
            # touch the Exp LUT once so the lazy activation-table load isn't
            # on the first score tile's critical path
            scr = kv.tile([P, 1], f32, tag="scr", name="scr")
            nc.scalar.activation(scr[:], ones_sb[:], Exp)
            mA_sb = [kv.tile([P, QT], bf16, tag=f"mA{k}", name=f"mA{k}")
                     for k in range(NKA)]
            mB_sb = [kv.tile([P, QT], bf16, tag=f"mB{k}", name=f"mB{k}")
                     for k in range(NKB - NKA)]

            # DRAM bounce buffers for the pair K/V exchange (split in two
            # per tensor so the CC stream pipelines and early keys land
            # early).  Region r of each agout holds group-rank r's half.
            agin_k = [dram.tile([D, QT], bf16, name=f"agin_k{i}")
                      for i in range(2)]
            agout_k = [dram.tile([2 * D, QT], bf16, name=f"agout_k{i}")
                       for i in range(2)]
            agin_v = [dram.tile([4 * P, D], bf16, name=f"agin_v{i}")
                      for i in range(2)]
            agout_v = [dram.tile([8 * P, D], bf16, name=f"agout_v{i}")
                       for i in range(2)]

            # ---- phase 1: load inputs + QKV projections -----------------
            with (
                tc.tile_pool(name="inp", bufs=1) as inp,
                tc.tile_pool(name="pps", bufs=2, space="PSUM") as pps,
            ):
                x_sb = [inp.tile([P, KO], bf16, tag=f"x{d}", name=f"x{d}")
                        for d in range(DCH)]
                xq_sb = [inp.tile([P, 2 * QT], bf16, tag=f"xq{d}", name=f"xq{d}")
                         for d in range(DCH)]
                wq_sb = [inp.tile([P, D], bf16, tag=f"wq{d}", name=f"wq{d}")
                         for d in range(DCH)]
                wk_sb = [inp.tile([P, D], bf16, tag=f"wk{d}", name=f"wk{d}")
                         for d in range(DCH)]
                wv_sb = [inp.tile([P, D], bf16, tag=f"wv{d}", name=f"wv{d}")
                        for d in range(DCH)]
                warm = inp.tile([P, QT], bf16, tag="warm", name="warm")

                # PE warm-up: HAM un-throttles after ~3.4us of sustained
                # matmul activity; burn the initial DMA window on junk
                # matmuls so the real ones run at 2.4 GHz from the start.
                nc.gpsimd.memset(warm[:], 0.0)
                wps = pps.tile([P, QT], f32, tag="pj0", name="wps")
                for i in range(WARMUP_MM):
                    nc.tensor.matmul(wps[:], warm[:, 0:P], warm[:],
                                     start=True, stop=True,
                                     skip_group_check=True)

                # Load order = first-use order on the sync queue (the
                # fast DMA issuer; engine-issued DMAs block their queue
                # until the transfer completes).  K projection runs
                # d-outer, so interleave x (key-half 0) with wk per
                # d-chunk: the first matmul group only needs ~0.4 MB.
                for d in range(DCH):
                    rows = slice(d * P, (d + 1) * P)
                    nc.sync.dma_start(out=x_sb[d][:, 0:QT],
                                      in_=xT[rows, 0:QT])
                    nc.sync.dma_start(out=wk_sb[d][:], in_=wkT[rows, :])
                for d in range(DCH):
                    rows = slice(d * P, (d + 1) * P)
                    nc.sync.dma_start(out=x_sb[d][:, QT:KO],
                                      in_=xT[rows, QT:KO])
                for d in range(DCH):
                    rows = slice(d * P, (d + 1) * P)
                    nc.sync.dma_start(out=wv_sb[d][:], in_=wvT[rows, :])

                # K^T for the 1024 own keys, half (512 keys) at a time so
                # each half's AllGather fires as early as possible.  Loop
                # d-outer with 4-wide e-groups accumulating in parallel
                # PSUM banks; the first group only waits on x[d0]/wk[d0].
                def k_proj_half(half):
                    hsl = slice(half * QT, (half + 1) * QT)
                    for eg in range(2):
                        es = range(eg * 4, eg * 4 + 4)
                        pss = [pps.tile([P, QT], f32, tag=f"pj{i}",
                                        name=f"kps{half}_{eg}_{i}")
                               for i in range(4)]
                        for d in range(DCH):
                            for i, e in enumerate(es):
                                nc.tensor.matmul(
                                    pss[i][:],
                                    wk_sb[d][:, e * P:(e + 1) * P],
                                    x_sb[d][:, hsl],
                                    start=(d == 0), stop=(d == DCH - 1),
                                    skip_group_check=True,
                                )
                        for i, e in enumerate(es):
                            nc.vector.tensor_copy(kt_sb[e][:, hsl], pss[i][:])
                    for e in range(DCH):
                        nc.sync.dma_start(out=agin_k[half][e * P:(e + 1) * P, :],
                                          in_=kt_sb[e][:, hsl])
                    nc.gpsimd.collective_compute(
                        "AllGather", mybir.AluOpType.bypass,
                        replica_groups=PAIRS,
                        ins=[agin_k[half][:]], outs=[agout_k[half][:]],
                    )

                # Read back BOTH regions of a gather: final key order is
                # global and identical on both ranks.  half h, region r ->
                # kt cols [r*1024 + h*512 : .. + 512).
                def k_readback(half):
                    for r in range(2):
                        csl = slice(r * KO + half * QT, r * KO + (half + 1) * QT)
                        for e in range(DCH):
                            nc.sync.dma_start(
                                out=kt_sb[e][:, csl],
                                in_=agout_k[half][r * D + e * P:
                                                  r * D + (e + 1) * P, :])

                def v_proj(t):
                    tsl = slice(t * P, (t + 1) * P)
                    pss = [pps.tile([P, QT], f32, tag=f"pj{eh}",
                                    name=f"vps{t}_{eh}") for eh in range(2)]
                    for d in range(DCH):
                        for eh in range(2):
                            nc.tensor.matmul(
                                pss[eh][:], x_sb[d][:, tsl],
                                wv_sb[d][:, eh * QT:(eh + 1) * QT],
                                start=(d == 0), stop=(d == DCH - 1),
                            )
                    for eh in range(2):
                        nc.vector.tensor_copy(
                            v_sb[t][:, eh * QT:(eh + 1) * QT], pss[eh][:])

                def v_gather_half(half):
                    for t in range(half * 4, half * 4 + 4):
                        nc.sync.dma_start(
                            out=agin_v[half][(t - half * 4) * P:
                                             (t - half * 4 + 1) * P, :],
                            in_=v_sb[t][:])
                    nc.gpsimd.collective_compute(
                        "AllGather", mybir.AluOpType.bypass,
                        replica_groups=PAIRS,
                        ins=[agin_v[half][:]], outs=[agout_v[half][:]],
                    )

                # readback: half h region r -> v_sb[r*8 + h*4 + i]
                def v_readback(half):
                    for r in range(2):
                        for i in range(4):
                            nc.sync.dma_start(
                                out=v_sb[r * 8 + half * 4 + i][:],
                                in_=agout_v[half][(r * 4 + i) * P:
                                                  (r * 4 + i + 1) * P, :])

                # Emission order staggers the sync-queue DMAs so each one's
                # semaphore wait resolves roughly when the queue reaches it
                # (a blocked DMA stalls everything behind it on its queue).
                k_proj_half(0)
                # remaining inputs enter the sync queue behind agin_k0 (so
                # they don't delay the first gather): wq/xq for Q proj
                # (needed ~60us in) and the persistent mask tiles (needed
                # at the first score chunk's DVE mask-add).
                for d in range(DCH):
                    rows = slice(d * P, (d + 1) * P)
                    nc.sync.dma_start(out=wq_sb[d][:], in_=wqT[rows, :])
                    nc.sync.dma_start(out=xq_sb[d][:], in_=xqT[rows, :])
                for k in range(NKA):
                    nc.sync.dma_start(out=mA_sb[k][:],
                                      in_=maskA[k * P:(k + 1) * P, :])
                for k in range(NKB - NKA):
                    nc.sync.dma_start(out=mB_sb[k][:],
                                      in_=maskB[k * P:(k + 1) * P, :])
                k_proj_half(1)
                k_readback(0)
                for t in range(4):
                    v_proj(t)
                v_gather_half(0)
                k_readback(1)
                for t in range(4, 8):
                    v_proj(t)
                v_gather_half(1)
                v_readback(0)

                # Q^T[e, q]: one wq weight tile drives both query halves.
                for e in range(DCH):
                    esl = slice(e * P, (e + 1) * P)
                    pss = [pps.tile([P, QT], f32, tag=f"pj{qh}",
                                    name=f"qps{e}_{qh}") for qh in range(2)]
                    for d in range(DCH):
                        for qh in range(2):
                            nc.tensor.matmul(
                                pss[qh][:], wq_sb[d][:, esl],
                                xq_sb[d][:, qh * QT:(qh + 1) * QT],
                                start=(d == 0), stop=(d == DCH - 1),
                            )
                    for qh in range(2):
                        nc.vector.tensor_copy(
                            qt_sb[e][:, qh * QT:(qh + 1) * QT], pss[qh][:])
                v_readback(1)

            # ---- phase 2: attention, one 512-query tile at a time -------
            with (
                tc.tile_pool(name="pp", bufs=2) as pp,
                tc.tile_pool(name="ost", bufs=4) as ost,
                tc.tile_pool(name="msc", bufs=2) as msc,
                tc.tile_pool(name="scp", bufs=4, space="PSUM") as scp,
                tc.tile_pool(name="aps", bufs=2, space="PSUM") as aps,
            ):
                qslA, qslB = slice(0, QT), slice(QT, 2 * QT)

                def exp_tile(t_idx, k, ps):
                    ph = pp.tile([P, QT], bf16, tag=f"p{k}",
                                 name=f"ph{t_idx}_{k}")
                    # exp((score + mask) / sqrt(1024)); no max subtraction
                    nc.scalar.activation(ph[:], ps[:], Exp, scale=0.03125)
                    return ph

                def recip_chain(t_idx, sps):
                    srow = msc.tile([1, QT], f32, tag="srow", name=f"srow{t_idx}")
                    nc.vector.tensor_copy(srow[:], sps[:])
                    bc = aps.tile([P, QT], f32, tag="sum", name=f"bc{t_idx}")
                    nc.tensor.matmul(bc[:], onesr[:, 0:P], srow[:],
                                     start=True, stop=True)
                    recb = msc.tile([P, QT], f32, tag="recb", name=f"recb{t_idx}")
                    nc.vector.reciprocal_approx_fast(out=recb[:], in_=bc[:])
                    return recb

                # Key chunks 0..7 are shared by both query tiles: interleave
                # their score matmuls so each KT slice is loaded into the PE
                # array once and drives two matmuls.  Tile B is fully valid
                # on chunks 0..7 (no mask); tile A is masked everywhere.
                phatA, phatB = [], []
                for k in range(NKA):
                    ksl = slice(k * P, (k + 1) * P)
                    psA = scp.tile([P, QT], f32, tag="sc", name=f"scA{k}")
                    psB = scp.tile([P, QT], f32, tag="sc", name=f"scB{k}")
                    for e in range(DCH):
                        nc.tensor.matmul(
                            psA[:], kt_sb[e][:, ksl], qt_sb[e][:, qslA],
                            start=(e == 0), stop=(e == DCH - 1),
                            skip_group_check=True,
                        )
                        nc.tensor.matmul(
                            psB[:], kt_sb[e][:, ksl], qt_sb[e][:, qslB],
                            start=(e == 0), stop=(e == DCH - 1),
                            skip_group_check=True,
                        )
                    nc.vector.tensor_tensor(psA[:], psA[:], mA_sb[k][:], op=add)
                    phatA.append(exp_tile(0, k, psA))
                    phatB.append(exp_tile(1, k, psB))

                # tile A's softmax denominator + 1/sum while tile B's upper
                # chunks are still streaming on PE
                spsA = aps.tile([1, QT], f32, tag="sum", name="sumA")
                for k in range(NKA):
                    nc.tensor.matmul(
                        spsA[:], ones_sb[:, 0:1], phatA[k][:],
                        start=(k == 0), stop=(k == NKA - 1),
                    )
                recbA = recip_chain(0, spsA)

                # tile B's private upper-half chunks (the only maskable ones)
                for k in range(NKA, NKB):
                    ksl = slice(k * P, (k + 1) * P)
                    psB = scp.tile([P, QT], f32, tag="sc", name=f"scB{k}")
                    for e in range(DCH):
                        nc.tensor.matmul(
                            psB[:], kt_sb[e][:, ksl], qt_sb[e][:, qslB],
                            start=(e == 0), stop=(e == DCH - 1),
                        )
                    nc.vector.tensor_tensor(psB[:], psB[:], mB_sb[k - NKA][:],
                                            op=add)
                    phatB.append(exp_tile(1, k, psB))

                spsB = aps.tile([1, QT], f32, tag="sum", name="sumB")
                for k in range(NKB):
                    nc.tensor.matmul(
                        spsB[:], ones_sb[:, 0:1], phatB[k][:],
                        start=(k == 0), stop=(k == NKB - 1),
                    )
                recbB = recip_chain(1, spsB)

                # AV, interleaved the same way: one V slice load drives both
                # tiles' accumulations for chunks 0..7.
                # k-order puts the V2-gather-dependent chunks (4..7,
                # 12..15) late so the AV never waits on the last readback
                AV_ORDER = [0, 1, 2, 3, 8, 9, 10, 11, 4, 5, 6, 7,
                            12, 13, 14, 15]
                for e in range(DCH):
                    esl = slice(e * P, (e + 1) * P)
                    psA = aps.tile([P, QT], f32, tag="av", name=f"avA{e}")
                    psB = aps.tile([P, QT], f32, tag="av", name=f"avB{e}")
                    for k in AV_ORDER:
                        if k < NKA:
                            nc.tensor.matmul(
                                psA[:], v_sb[k][:, esl], phatA[k][:],
                                start=(k == 0), stop=(k == NKA - 1),
                                skip_group_check=True,
                            )
                        nc.tensor.matmul(
                            psB[:], v_sb[k][:, esl], phatB[k][:],
                            start=(k == 0), stop=(k == NKB - 1),
                            skip_group_check=True,
                        )
                    assert AV_ORDER[-1] == NKB - 1 and AV_ORDER[8 + 3] == NKA - 1
                    for t_idx, ps, recb, qsl in ((0, psA, recbA, qslA),
                                                 (1, psB, recbB, qslB)):
                        ot = ost.tile([P, QT], f32, tag="ot",
                                      name=f"ot{t_idx}_{e}")
                        nc.vector.tensor_tensor(ot[:], ps[:], recb[:], op=mult)
                        nc.sync.dma_start(out=outT[esl, qsl], in_=ot[:])

    nc.compile()
    return nc


# h=0 -> query blocks [0:512) (tile A) and [1536:2048) (tile B)
# h=1 -> query blocks [512:1024) (tile A) and [1024:1536) (tile B)
_QSTARTS = ((0, 3 * QT), (QT, 2 * QT))


def _make_mask(q0: int, ctx: int) -> np.ndarray:
    k = np.arange(ctx)[:, None]
    q = q0 + np.arange(QT)[None, :]
    return np.where(k <= q, np.float32(0.0), NEG).astype(_BF16)


def _in_maps(x, Wk, Wq, Wv):
    wq_t = np.ascontiguousarray(Wk.T.astype(_BF16))   # ref swap: q uses Wk
    wk_t = np.ascontiguousarray(Wq.T.astype(_BF16))
    wv_t = np.ascontiguousarray(Wv.T.astype(_BF16))
    maps = []
    for c in range(8):
        b, h = divmod(c, 2)
        qa, qb = _QSTARTS[h]
        xb = x[b].astype(_BF16)
        # own keys for the K/V projection split
        x_t = np.ascontiguousarray(xb[h * KO:(h + 1) * KO].T)
        xq_t = np.ascontiguousarray(
            np.concatenate([xb[qa:qa + QT], xb[qb:qb + QT]], axis=0).T
        )
        maps.append({
            "xT": x_t,
            "xqT": xq_t,
            "wqT": wq_t,
            "wkT": wk_t,
            "wvT": wv_t,
            "maskA": _make_mask(qa, CTX_A),
            # tile B chunks 0..7 are fully valid on every core; only the
            # upper half of its context can need masking
            "maskB": _make_mask(qb, CTX_B)[CTX_B // 2:],
        })
    return maps


def _assemble(results):
    out = np.empty((B, S, D), dtype=np.float32)
    for c, res in enumerate(results):
        b, h = divmod(c, 2)
        qa, qb = _QSTARTS[h]
        o = res["outT"]
        out[b, qa:qa + QT] = o[:, 0:QT].T
        out[b, qb:qb + QT] = o[:, QT:2 * QT].T
    return out


def kernel(x, Wk, Wq, Wv, _trace=False):
    from concourse.bass_utils import run_bass_kernel_spmd

    nc = _build_nc()
    res = run_bass_kernel_spmd(nc, _in_maps(x, Wk, Wq, Wv), list(range(8)),
                               trace=_trace)
    out = _assemble(res.results)
    if _trace:
        return out, res
    return out


# revision 15
# speedup vs baseline: 1.4175x; 1.1073x over previous
"""Causal single-head attention on 8 TRN2 NeuronCores.

Problem (hardcoded): x [4, 2048, 1024] f32; Wk, Wq, Wv [1024, 1024] f32.
  q = x @ Wk.T ; k = x @ Wq.T ; v = x @ Wv.T        (note ref's q/k weight swap)
  out = softmax(mask(q @ k.T) / sqrt(1024)) @ v

Sharding: 2 cores per batch.  Core h of a batch owns four 256-query strips,
processed in "slots" with a fixed causal context template {512, 1024, 1536,
2048} keys: h=0 owns strips {0,3,4,7} (q0 = 0, 768, 1024, 1792), h=1 owns
{1,2,5,6} (256, 512, 1280, 1536) — every core runs the identical program
(true SPMD); the slot template dominates each core's per-strip causal needs
and per-core additive masks (only the last 4 key-chunks of each slot can be
non-trivial: diagonal or template padding) encode causality exactly.

K/V projection is FULLY split across the pair: core h projects K^T and V
only for its own 1024 keys (global keys [h*1024:(h+1)*1024)) and the halves
are exchanged through DRAM bounce buffers with three pair AllGathers (K in
two 512-key pieces for early availability, V in one) — collective_compute
blocks its engine until completion, so the ops serialize on the CC stream;
triggers are arranged so the chain still lands each tensor before first
use.  Own-key projection output is staged in the low half of the K^T / V
SBUF tensors; the gather readback (both regions, so the final key order is
global and identical on both ranks) overwrites them.

On-chip layout is feature-major (all host-side transposes are free):
  xT/wT in, Q^T/K^T feature-major, V sequence-major.  Scores are computed
  as S^T[k, q] so softmax needs no on-chip transpose anywhere: the score
  loop runs key-chunk-outer so one KT slice load feeds up to four slots'
  matmuls; exp via ACT (no max subtraction -- scaled scores are ~N(0,1)),
  sum-of-exp via a ones-column matmul per slot as soon as its chunks are
  done, AV runs e-outer/key-inner so one V slice load feeds up to four
  slots, accumulating out^T[e, q].  The per-query 1/sum is broadcast
  across partitions with a K=1 PE matmul and applied by DVE during the
  PSUM->SBUF output copy.  Output returns as out^T in slot order and is
  scattered back on the host.  All matmuls bf16 with fp32 PSUM
  accumulation.

The kernel opens with warm-up matmuls on a zeroed tile so the PE HAM
clock-gate reaches 8/8 (2.4 GHz) while the first input DMAs are in flight;
the K projection runs d-outer so its first matmul only needs ~0.4 MB of
DMA.  DMA queues: streaming loads and exchange DMAs all go on the sync
queue, emitted in the order their semaphore waits resolve (a blocked DMA
head-of-line-blocks its queue).
"""

import functools

import ml_dtypes
import numpy as np

B = 4
S = 2048
D = 1024
P = 128
DCH = D // P            # 8 contraction chunks
QT = 512                # projection column-tile width
QS = 256                # query-strip width (phase 2)
KO = 1024               # own keys per core (projection split)
NKB = S // P            # 16 key chunks
TPL = (4, 8, 12, 16)    # context template per slot, in 128-key chunks
NEG = np.float32(-30000.0)
WARMUP_MM = 18

_BF16 = ml_dtypes.bfloat16

# slot -> strip start q0, per h  (strip req <= 128*TPL[slot])
_QSTARTS = ((0, 768, 1024, 1792), (256, 512, 1280, 1536))


@functools.lru_cache(maxsize=1)
def _build_nc():
    import concourse.bass as bass  # noqa: F401  (registers engines)
    import concourse.mybir as mybir
    from concourse import bacc, tile

    bf16 = mybir.dt.bfloat16
    f32 = mybir.dt.float32
    add = mybir.AluOpType.add
    mult = mybir.AluOpType.mult
    Exp = mybir.ActivationFunctionType.Exp
    PAIRS = [[2 * i, 2 * i + 1] for i in range(4)]

    nc = bacc.Bacc("TRN2", target_bir_lowering=False, debug=False, num_devices=8)

    xT = nc.declare_dram_parameter("xT", [D, KO], bf16, isOutput=False)
    xqT = nc.declare_dram_parameter("xqT", [D, 4 * QS], bf16, isOutput=False)
    wqT = nc.declare_dram_parameter("wqT", [D, D], bf16, isOutput=False)
    wkT = nc.declare_dram_parameter("wkT", [D, D], bf16, isOutput=False)
    wvT = nc.declare_dram_parameter("wvT", [D, D], bf16, isOutput=False)
    masks = nc.declare_dram_parameter("masks", [16 * P, QS], bf16,
                                      isOutput=False)
    outT = nc.declare_dram_parameter("outT", [D, 4 * QS], f32, isOutput=True)

    with tile.TileContext(nc) as tc:
        with (
            tc.tile_pool(name="kv", bufs=1) as kv,
            tc.tile_pool(name="dram", bufs=1, space="DRAM") as dram,
        ):
            # ---- persistent SBUF tensors --------------------------------
            kt_sb = [kv.tile([P, S], bf16, tag=f"kt{e}", name=f"kt{e}")
                     for e in range(DCH)]
            qt_sb = [kv.tile([P, 4 * QS], bf16, tag=f"qt{e}", name=f"qt{e}")
                     for e in range(DCH)]
            v_sb = [kv.tile([P, D], bf16, tag=f"v{t}", name=f"v{t}")
                    for t in range(NKB)]
            ones_sb = kv.tile([P, 1], bf16, tag="ones", name="ones")
            nc.gpsimd.memset(ones_sb[:], 1.0)
            onesr = kv.tile([1, P], f32, tag="onesr", name="onesr")
            nc.gpsimd.memset(onesr[:], 1.0)
            # touch the Exp LUT once so the lazy activation-table load isn't
            # on the first score tile's critical path
            scr = kv.tile([P, 1], f32, tag="scr", name="scr")
            nc.scalar.activation(scr[:], ones_sb[:], Exp)
            # mask tiles: slot s, local chunk j (= key chunk TPL[s]-4+j)
            msk_sb = [[kv.tile([P, QS], bf16, tag=f"m{s}_{j}",
                               name=f"m{s}_{j}") for j in range(4)]
                      for s in range(4)]

            # DRAM bounce buffers for the pair K/V exchange.  K goes in two
            # 512-key pieces so early key chunks land early; V in one (the
            # blocking CC ops serialize anyway and V is needed last).
            # Region r of each agout holds group-rank r's piece.
            agin_k = [dram.tile([D, QT], bf16, name=f"agin_k{i}")
                      for i in range(2)]
            agout_k = [dram.tile([2 * D, QT], bf16, name=f"agout_k{i}")
                       for i in range(2)]
            agin_v = dram.tile([8 * P, D], bf16, name="agin_v")
            agout_v = dram.tile([16 * P, D], bf16, name="agout_v")

            # ---- phase 1: load inputs + QKV projections -----------------
            with (
                tc.tile_pool(name="inp", bufs=1) as inp,
                tc.tile_pool(name="pps", bufs=2, space="PSUM") as pps,
            ):
                x_sb = [inp.tile([P, KO], bf16, tag=f"x{d}", name=f"x{d}")
                        for d in range(DCH)]
                xq_sb = [inp.tile([P, 4 * QS], bf16, tag=f"xq{d}",
                                  name=f"xq{d}") for d in range(DCH)]
                wq_sb = [inp.tile([P, D], bf16, tag=f"wq{d}", name=f"wq{d}")
                         for d in range(DCH)]
                wk_sb = [inp.tile([P, D], bf16, tag=f"wk{d}", name=f"wk{d}")
                         for d in range(DCH)]
                wv_sb = [inp.tile([P, D], bf16, tag=f"wv{d}", name=f"wv{d}")
                        for d in range(DCH)]
                warm = inp.tile([P, QT], bf16, tag="warm", name="warm")

                # PE warm-up: HAM un-throttles after ~3.4us of sustained
                # matmul activity; burn the initial DMA window on junk
                # matmuls so the real ones run at 2.4 GHz from the start.
                nc.gpsimd.memset(warm[:], 0.0)
                wps = pps.tile([P, QT], f32, tag="pj0", name="wps")
                for i in range(WARMUP_MM):
                    nc.tensor.matmul(wps[:], warm[:, 0:P], warm[:],
                                     start=True, stop=True,
                                     skip_group_check=True)

                # Streaming loads in first-use order on the sync queue.  K
                # projection runs d-outer, so interleave x (key-half 0)
                # with wk per d-chunk: the first matmul group only needs
                # ~0.4 MB.  wv and later loads are emitted after
                # k_proj_half(0) so agin_k0 enters the queue early.
                for d in range(DCH):
                    rows = slice(d * P, (d + 1) * P)
                    nc.sync.dma_start(out=x_sb[d][:, 0:QT],
                                      in_=xT[rows, 0:QT])
                    nc.sync.dma_start(out=wk_sb[d][:], in_=wkT[rows, :])
                for d in range(DCH):
                    rows = slice(d * P, (d + 1) * P)
                    nc.sync.dma_start(out=x_sb[d][:, QT:KO],
                                      in_=xT[rows, QT:KO])

                # K^T for the 1024 own keys, half (512 keys) at a time so
                # each half's AllGather fires as early as possible.  Loop
                # d-outer with 4-wide e-groups accumulating in parallel
                # PSUM banks; the first group only waits on x[d0]/wk[d0].
                def k_proj_half(half):
                    hsl = slice(half * QT, (half + 1) * QT)
                    for eg in range(2):
                        es = range(eg * 4, eg * 4 + 4)
                        pss = [pps.tile([P, QT], f32, tag=f"pj{i}",
                                        name=f"kps{half}_{eg}_{i}")
                               for i in range(4)]
                        for d in range(DCH):
                            for i, e in enumerate(es):
                                nc.tensor.matmul(
                                    pss[i][:],
                                    wk_sb[d][:, e * P:(e + 1) * P],
                                    x_sb[d][:, hsl],
                                    start=(d == 0), stop=(d == DCH - 1),
                                    skip_group_check=True,
                                )
                        for i, e in enumerate(es):
                            nc.vector.tensor_copy(kt_sb[e][:, hsl], pss[i][:])
                    for e in range(DCH):
                        nc.sync.dma_start(out=agin_k[half][e * P:(e + 1) * P, :],
                                          in_=kt_sb[e][:, hsl])
                    nc.gpsimd.collective_compute(
                        "AllGather", mybir.AluOpType.bypass,
                        replica_groups=PAIRS,
                        ins=[agin_k[half][:]], outs=[agout_k[half][:]],
                    )

                # Read back BOTH regions of a gather: final key order is
                # global and identical on both ranks.  half h, region r ->
                # kt cols [r*1024 + h*512 : .. + 512).
                def k_readback(half):
                    for r in range(2):
                        csl = slice(r * KO + half * QT, r * KO + (half + 1) * QT)
                        for e in range(DCH):
                            nc.sync.dma_start(
                                out=kt_sb[e][:, csl],
                                in_=agout_k[half][r * D + e * P:
                                                  r * D + (e + 1) * P, :])

                def v_proj(t):
                    tsl = slice(t * P, (t + 1) * P)
                    pss = [pps.tile([P, QT], f32, tag=f"pj{eh}",
                                    name=f"vps{t}_{eh}") for eh in range(2)]
                    for d in range(DCH):
                        for eh in range(2):
                            nc.tensor.matmul(
                                pss[eh][:], x_sb[d][:, tsl],
                                wv_sb[d][:, eh * QT:(eh + 1) * QT],
                                start=(d == 0), stop=(d == DCH - 1),
                            )
                    for eh in range(2):
                        nc.vector.tensor_copy(
                            v_sb[t][:, eh * QT:(eh + 1) * QT], pss[eh][:])

                # Emission order staggers the sync-queue DMAs so each one's
                # semaphore wait resolves roughly when the queue reaches it
                # (a blocked DMA stalls everything behind it on its queue).
                k_proj_half(0)
                for d in range(DCH):
                    rows = slice(d * P, (d + 1) * P)
                    nc.sync.dma_start(out=wv_sb[d][:], in_=wvT[rows, :])
                for d in range(DCH):
                    rows = slice(d * P, (d + 1) * P)
                    nc.sync.dma_start(out=wq_sb[d][:], in_=wqT[rows, :])
                    nc.sync.dma_start(out=xq_sb[d][:], in_=xqT[rows, :])
                for s in range(4):
                    for j in range(4):
                        rows = slice((4 * s + j) * P, (4 * s + j + 1) * P)
                        nc.sync.dma_start(out=msk_sb[s][j][:],
                                          in_=masks[rows, :])
                k_proj_half(1)
                k_readback(0)
                for t in range(DCH):
                    v_proj(t)
                for t in range(DCH):
                    nc.sync.dma_start(out=agin_v[t * P:(t + 1) * P, :],
                                      in_=v_sb[t][:])
                nc.gpsimd.collective_compute(
                    "AllGather", mybir.AluOpType.bypass,
                    replica_groups=PAIRS,
                    ins=[agin_v[:]], outs=[agout_v[:]],
                )
                k_readback(1)

                # Q^T[e, q]: one wq weight tile drives two query halves.
                for e in range(DCH):
                    esl = slice(e * P, (e + 1) * P)
                    pss = [pps.tile([P, QT], f32, tag=f"pj{qh}",
                                    name=f"qps{e}_{qh}") for qh in range(2)]
                    for d in range(DCH):
                        for qh in range(2):
                            nc.tensor.matmul(
                                pss[qh][:], wq_sb[d][:, esl],
                                xq_sb[d][:, qh * QT:(qh + 1) * QT],
                                start=(d == 0), stop=(d == DCH - 1),
                            )
                    for qh in range(2):
                        nc.vector.tensor_copy(
                            qt_sb[e][:, qh * QT:(qh + 1) * QT], pss[qh][:])

                # V readback: region r chunk i -> v_sb[r*8 + i]
                for r in range(2):
                    for i in range(DCH):
                        nc.sync.dma_start(
                            out=v_sb[r * 8 + i][:],
                            in_=agout_v[(r * 8 + i) * P:(r * 8 + i + 1) * P, :])

            # ---- phase 2: attention over four 256-query slots -----------
            # Slots are processed in PAIRS (0,1) and (2,3): for key chunks
            # both slots of a pair need, ONE N=512 matmul covers both
            # (their queries are adjacent in qt).  A pair shares PSUM
            # banks; the upper slot's private chunks continue accumulating
            # into the bank's high half with start=False (has_written bits
            # are already set -- start=True would clear the WHOLE bank and
            # wipe the partner's partials).
            with (
                tc.tile_pool(name="pp", bufs=1) as pp,
                tc.tile_pool(name="ost", bufs=4) as ost,
                tc.tile_pool(name="msc", bufs=2) as msc,
                tc.tile_pool(name="scp", bufs=2, space="PSUM") as scp,
                tc.tile_pool(name="sol", bufs=2, space="PSUM") as sol,
                tc.tile_pool(name="smp", bufs=2, space="PSUM") as smp,
            ):
                TLO = (TPL[0], TPL[2])   # pair g: low slot = 2g, high = 2g+1
                THI = (TPL[1], TPL[3])
                phat = {}   # (g, k) -> [P, 2*QS] (k < TLO) or [P, QS]
                recb = {}   # pair -> [P, 2*QS] f32 broadcast reciprocal

                def pair_sum(g):
                    sps = smp.tile([1, 2 * QS], f32, tag="sm", name=f"sum{g}")
                    for k in range(THI[g]):
                        if k < TLO[g]:
                            nc.tensor.matmul(
                                sps[:], ones_sb[:, 0:1], phat[g, k][:],
                                start=(k == 0), stop=False,
                                skip_group_check=True,
                            )
                        else:
                            nc.tensor.matmul(
                                sps[:, QS:2 * QS], ones_sb[:, 0:1],
                                phat[g, k][:],
                                start=False, stop=(k == THI[g] - 1),
                                skip_group_check=True,
                            )
                    srow = msc.tile([1, 2 * QS], f32, tag="srow", name=f"srow{g}")
                    nc.vector.tensor_copy(srow[:], sps[:])
                    bc = smp.tile([P, 2 * QS], f32, tag="sm", name=f"bc{g}")
                    nc.tensor.matmul(bc[:], onesr[:, 0:P], srow[:],
                                     start=True, stop=True)
                    rb = msc.tile([P, 2 * QS], f32, tag=f"recb{g}",
                                  name=f"recb{g}")
                    nc.vector.reciprocal_approx_fast(out=rb[:], in_=bc[:])
                    recb[g] = rb

                # scores, key-chunk-outer: one kt slice load drives both
                # pairs' matmuls.  Only the last 4 chunks of each slot can
                # need masking (diagonal or template padding); for the low
                # slot those fall in the pair phase (mask the low half),
                # for the high slot in its solo phase.
                for k in range(NKB):
                    ksl = slice(k * P, (k + 1) * P)
                    ps = {}
                    for g in range(2):
                        if k < TLO[g]:
                            ps[g] = scp.tile([P, 2 * QS], f32, tag=f"pg{g}",
                                             name=f"pg{g}_{k}")
                        elif k < THI[g]:
                            ps[g] = sol.tile([P, QS], f32, tag="so",
                                             name=f"so{g}_{k}")
                    for e in range(DCH):
                        for g in range(2):
                            if k < TLO[g]:
                                nc.tensor.matmul(
                                    ps[g][:], kt_sb[e][:, ksl],
                                    qt_sb[e][:, 2 * g * QS:(2 * g + 2) * QS],
                                    start=(e == 0), stop=(e == DCH - 1),
                                    skip_group_check=True,
                                )
                            elif k < THI[g]:
                                nc.tensor.matmul(
                                    ps[g][:], kt_sb[e][:, ksl],
                                    qt_sb[e][:, (2 * g + 1) * QS:
                                               (2 * g + 2) * QS],
                                    start=(e == 0), stop=(e == DCH - 1),
                                    skip_group_check=True,
                                )
                    for g in range(2):
                        if k < TLO[g]:
                            j = k - (TLO[g] - 4)
                            if j >= 0:
                                nc.vector.tensor_tensor(
                                    ps[g][:, 0:QS], ps[g][:, 0:QS],
                                    msk_sb[2 * g][j][:], op=add)
                            ph = pp.tile([P, 2 * QS], bf16, tag=f"pp{g}_{k}",
                                         name=f"php{g}_{k}")
                            nc.scalar.activation(ph[:], ps[g][:], Exp,
                                                 scale=0.03125)
                            phat[g, k] = ph
                        elif k < THI[g]:
                            j = k - (THI[g] - 4)
                            if j >= 0:
                                nc.vector.tensor_tensor(
                                    ps[g][:], ps[g][:],
                                    msk_sb[2 * g + 1][j][:], op=add)
                            ph = pp.tile([P, QS], bf16, tag=f"po{g}_{k}",
                                         name=f"pho{g}_{k}")
                            nc.scalar.activation(ph[:], ps[g][:], Exp,
                                                 scale=0.03125)
                            phat[g, k] = ph
                        if k == THI[g] - 1:
                            pair_sum(g)

                # AV, e-outer / key-chunk-inner: one V slice load drives
                # both pairs.  out^T[e, q], two slots per PSUM bank.
                for e in range(DCH):
                    esl = slice(e * P, (e + 1) * P)
                    av = [scp.tile([P, 2 * QS], f32, tag=f"pg{g}",
                                   name=f"avg{g}_{e}") for g in range(2)]
                    for k in range(NKB):
                        for g in range(2):
                            if k < TLO[g]:
                                nc.tensor.matmul(
                                    av[g][:], v_sb[k][:, esl], phat[g, k][:],
                                    start=(k == 0), stop=False,
                                    skip_group_check=True,
                                )
                            elif k < THI[g]:
                                nc.tensor.matmul(
                                    av[g][:, QS:2 * QS], v_sb[k][:, esl],
                                    phat[g, k][:],
                                    start=False, stop=(k == THI[g] - 1),
                                    skip_group_check=True,
                                )
                    for g in range(2):
                        ot = ost.tile([P, 2 * QS], f32, tag="ot",
                                      name=f"ot{g}_{e}")
                        nc.vector.tensor_tensor(ot[:], av[g][:], recb[g][:],
                                                op=mult)
                        nc.sync.dma_start(
                            out=outT[esl, 2 * g * QS:(2 * g + 2) * QS],
                            in_=ot[:])

    nc.compile()
    return nc


def _make_masks(h: int) -> np.ndarray:
    """[16*128, 256] bf16: slot s rows [4s*128:(4s+4)*128) = key chunks
    TPL[s]-4 .. TPL[s]-1 vs that slot's 256 queries."""
    m = np.empty((16 * P, QS), dtype=np.float32)
    for s in range(4):
        q0 = _QSTARTS[h][s]
        q = q0 + np.arange(QS)[None, :]
        for j in range(4):
            c = TPL[s] - 4 + j
            kk = c * P + np.arange(P)[:, None]
            m[(4 * s + j) * P:(4 * s + j + 1) * P] = np.where(kk <= q, 0.0, NEG)
    return m.astype(_BF16)


def _in_maps(x, Wk, Wq, Wv):
    wq_t = np.ascontiguousarray(Wk.T.astype(_BF16))   # ref swap: q uses Wk
    wk_t = np.ascontiguousarray(Wq.T.astype(_BF16))
    wv_t = np.ascontiguousarray(Wv.T.astype(_BF16))
    mby_h = [_make_masks(0), _make_masks(1)]
    maps = []
    for c in range(8):
        b, h = divmod(c, 2)
        xb = x[b].astype(_BF16)
        # own keys for the K/V projection split
        x_t = np.ascontiguousarray(xb[h * KO:(h + 1) * KO].T)
        xq_t = np.ascontiguousarray(
            np.concatenate([xb[q0:q0 + QS] for q0 in _QSTARTS[h]], axis=0).T
        )
        maps.append({
            "xT": x_t,
            "xqT": xq_t,
            "wqT": wq_t,
            "wkT": wk_t,
            "wvT": wv_t,
            "masks": mby_h[h],
        })
    return maps


def _assemble(results):
    out = np.empty((B, S, D), dtype=np.float32)
    for c, res in enumerate(results):
        b, h = divmod(c, 2)
        o = res["outT"]
        for s, q0 in enumerate(_QSTARTS[h]):
            out[b, q0:q0 + QS] = o[:, s * QS:(s + 1) * QS].T
    return out


def kernel(x, Wk, Wq, Wv, _trace=False):
    from concourse.bass_utils import run_bass_kernel_spmd

    nc = _build_nc()
    res = run_bass_kernel_spmd(nc, _in_maps(x, Wk, Wq, Wv), list(range(8)),
                               trace=_trace)
    out = _assemble(res.results)
    if _trace:
        return out, res
    return out
